# revision 10
# baseline (speedup 1.0000x reference)
"""Trainium2 Bass kernel for nn_AttentionBlock (GroupNorm + 8-head self-attention).

Data-parallel over batch: 8 batch elements -> 8 NeuronCores, one each.

Per-core layout ([c, n] with c on partitions, c = 4 chunks x 128, n = 1024):
  GroupNorm   : bn_stats per channel -> group-combine matmul (G) ->
                broadcast matmul (G^T) -> fused scale/shift -> xn (fp32r)
  QKV         : Q,K in [o, n] layout (lhsT = wqkvT chunks, fp32r),
                V in [n, o] layout (V^T, lhsT = xn chunks)
  Attention   : per head-pair p (heads 2p, 2p+1):
                  S^T[m,n] = K^T Q  (bf16, 2 heads row-packed via tile_position)
                  P = exp(S/8)      (ScalarE, PSUM->SBUF, bf16)
                  att_un = V^T.T @ P (bf16, 2 heads col-packed, M=64 each)
                  rowsums via ones-vector matmuls (M=1, 4 col positions)
                  normalize: att = att_un * (1/rowsum) broadcast via GpSimd
  Proj        : fp32r matmul + residual add, output fp32
"""

import numpy as np

NUM_GROUPS = 32
NUM_HEADS = 8
EPS = 1e-6
C = 512
N = 1024
B = 8

_cache = {}


def _build_bass(debug=False):
    import concourse.bacc as bacc
    import concourse.mybir as mybir
    import concourse.tile as tile

    fp32 = mybir.dt.float32
    fp32r = mybir.dt.float32r
    bf16 = mybir.dt.bfloat16
    AF = mybir.ActivationFunctionType
    OP = mybir.AluOpType
    AX = mybir.AxisListType

    nc = bacc.Bacc("TRN2", target_bir_lowering=False, debug=False)

    x_d = nc.dram_tensor("x", [C, N], fp32, kind="ExternalInput")
    wqkvT_d = nc.dram_tensor("wqkvT", [C, 3 * C], fp32, kind="ExternalInput")
    projT_d = nc.dram_tensor("projT", [C, C], fp32, kind="ExternalInput")
    qkvb_d = nc.dram_tensor("qkv_b", [3 * C], fp32, kind="ExternalInput")
    projb_d = nc.dram_tensor("proj_b", [C], fp32, kind="ExternalInput")
    nw_d = nc.dram_tensor("norm_w", [C], fp32, kind="ExternalInput")
    nb_d = nc.dram_tensor("norm_b", [C], fp32, kind="ExternalInput")
    G_d = nc.dram_tensor("Gmat", [128, 4, 32], fp32, kind="ExternalInput")
    GT_d = nc.dram_tensor("GTmat", [32, 4, 128], fp32, kind="ExternalInput")
    y_d = nc.dram_tensor("y", [C, N], fp32, kind="ExternalOutput")
    r8r_d = nc.dram_tensor("r8r_scratch", [4, 4, 512], fp32)
    if debug:
        dbg_xn = nc.dram_tensor("dbg_xn", [128, 4, N], fp32, kind="ExternalOutput")
        dbg_qk = nc.dram_tensor("dbg_qk", [128, 8, N], mybir.dt.uint16, kind="ExternalOutput")
        dbg_vt = nc.dram_tensor("dbg_vt", [128, 8, C], mybir.dt.uint16, kind="ExternalOutput")
        dbg_p0 = nc.dram_tensor("dbg_p0", [128, 8, 2048], mybir.dt.uint16, kind="ExternalOutput")
        dbg_rr = nc.dram_tensor("dbg_rr", [128, 4, 512], fp32, kind="ExternalOutput")
        dbg_att = nc.dram_tensor("dbg_att", [128, 4, N], fp32, kind="ExternalOutput")

    with tile.TileContext(nc) as tc:
        with (
            tc.tile_pool(name="const", bufs=1) as const,
            tc.tile_pool(name="work", bufs=1) as work,
            tc.tile_pool(name="ppool", bufs=1) as ppool,
            tc.tile_pool(name="rot", bufs=2) as rot,
            tc.tile_pool(name="pss", bufs=2, space="PSUM") as pss,     # 2x[128,1024]
            tc.tile_pool(name="psav", bufs=1, space="PSUM") as psav,   # [128,1024]
            tc.tile_pool(name="psr", bufs=1, space="PSUM") as psr,     # [128,512]
            tc.tile_pool(name="psg", bufs=1, space="PSUM") as psg,     # [128,512]
        ):
            # ---------------- load inputs ----------------
            x_sb = work.tile([128, 4, N], fp32, tag="x")
            nc.sync.dma_start(x_sb[:], x_d.ap().rearrange("(j p) n -> p j n", p=128))

            w_r = work.tile([128, 4, 3 * C], fp32r, tag="wr")
            p_r = work.tile([128, 4, C], fp32r, tag="pr")
            for j in range(4):
                stg = rot.tile([128, 3 * C], fp32, tag="stage")
                nc.sync.dma_start(
                    stg[:], wqkvT_d.ap().rearrange("(j p) o -> j p o", p=128)[j]
                )
                nc.vector.tensor_copy(w_r[:, j, :], stg[:])
            for j in range(4):
                stg = rot.tile([128, C], fp32, tag="stage2")
                nc.sync.dma_start(
                    stg[:], projT_d.ap().rearrange("(j p) o -> j p o", p=128)[j]
                )
                nc.vector.tensor_copy(p_r[:, j, :], stg[:])

            G_sb = const.tile([128, 4, 32], fp32, tag="G")
            GT_sb = const.tile([32, 4, 128], fp32, tag="GT")
            nc.sync.dma_start(G_sb[:], G_d.ap())
            nc.sync.dma_start(GT_sb[:], GT_d.ap())
            nw_sb = const.tile([128, 4], fp32, tag="nw")
            nb_sb = const.tile([128, 4], fp32, tag="nb")
            nc.sync.dma_start(nw_sb[:], nw_d.ap().rearrange("(j p) -> p j", p=128))
            nc.sync.dma_start(nb_sb[:], nb_d.ap().rearrange("(j p) -> p j", p=128))
            qb_sb = const.tile([128, 8], fp32, tag="qb")
            nc.sync.dma_start(
                qb_sb[:], qkvb_d.ap()[0 : 2 * C].rearrange("(o p) -> p o", p=128)
            )
            pb_sb = const.tile([128, 4], fp32, tag="pb")
            nc.sync.dma_start(pb_sb[:], projb_d.ap().rearrange("(j p) -> p j", p=128))
            import concourse.bass as bass_mod
            vb_src = qkvb_d.ap()[2 * C : 3 * C]
            vb_bcast_ap = bass_mod.AP(
                tensor=vb_src.tensor,
                offset=vb_src.offset,
                ap=[[0, 128], [1, C]],
            )
            vb_bc = const.tile([128, C], fp32, tag="vbbc")
            nc.sync.dma_start(vb_bc[:], vb_bcast_ap)
            ones_bf = const.tile([128, 1], bf16, tag="ones")
            nc.vector.memset(ones_bf[:], 1.0)

            # ---------------- groupnorm ----------------
            stats = work.tile([128, 4, 2, 6], fp32, tag="stats")
            for j in range(4):
                for u in range(2):
                    nc.vector.bn_stats(
                        stats[:, j, u, :], x_sb[:, j, u * 512 : u * 512 + 512]
                    )
            mv = work.tile([128, 4, 2], fp32, tag="mv")
            for j in range(4):
                nc.vector.bn_aggr(mv[:, j, :], stats[:, j, :, :])
            # ssq = (mean, E[x^2]) per channel
            ssq = work.tile([128, 4, 2], fp32, tag="ssq")
            nc.vector.tensor_copy(ssq[:, :, 0], mv[:, :, 0])
            nc.vector.tensor_tensor(
                ssq[:, :, 1], mv[:, :, 0], mv[:, :, 0], op=OP.mult
            )
            nc.vector.tensor_tensor(
                ssq[:, :, 1], ssq[:, :, 1], mv[:, :, 1], op=OP.add
            )
            # group stats [32, 2] = (mu_g, E[x^2]_g); G has 1/16 entries
            ps_g = psg.tile([32, 2], fp32, tag="g")
            for j in range(4):
                nc.tensor.matmul(
                    ps_g[:], G_sb[:, j, :], ssq[:, j, :], start=(j == 0), stop=(j == 3)
                )
            st2 = work.tile([32, 2], fp32, tag="st2")
            # var = E[x^2] - mu^2 -> rstd = 1/sqrt(var+eps); st2 = (mu, rstd)
            nc.vector.tensor_copy(st2[:, 0:1], ps_g[:, 0:1])
            var = work.tile([32, 1], fp32, tag="var")
            nc.vector.tensor_tensor(var[:], st2[:, 0:1], st2[:, 0:1], op=OP.mult)
            nc.vector.tensor_tensor(var[:], ps_g[:, 1:2], var[:], op=OP.subtract)
            eps_sb = const.tile([32, 1], fp32, tag="eps")
            nc.vector.memset(eps_sb[:], float(EPS))
            nc.scalar.activation(var[:], var[:], AF.Sqrt, bias=eps_sb[:], scale=1.0)
            nc.vector.reciprocal(st2[:, 1:2], var[:])
            # broadcast back to channels: [128, 2] per chunk
            ps_bc = psg.tile([128, 4, 2], fp32, tag="g")
            for j in range(4):
                nc.tensor.matmul(
                    ps_bc[:, j, :], GT_sb[:, j, :], st2[:], start=True, stop=True
                )
            # alpha = rstd*w ; beta = b - mu*alpha
            ab = work.tile([128, 4, 2], fp32, tag="ab")
            nc.vector.tensor_tensor(ab[:, :, 0], ps_bc[:, :, 1], nw_sb[:], op=OP.mult)
            nc.vector.tensor_tensor(ab[:, :, 1], ps_bc[:, :, 0], ab[:, :, 0], op=OP.mult)
            nc.vector.tensor_tensor(ab[:, :, 1], nb_sb[:], ab[:, :, 1], op=OP.subtract)

            xn_r = work.tile([128, 4, N], fp32r, tag="xn")
            for j in range(4):
                nc.vector.tensor_scalar(
                    xn_r[:, j, :],
                    x_sb[:, j, :],
                    ab[:, j, 0:1],
                    ab[:, j, 1:2],
                    op0=OP.mult,
                    op1=OP.add,
                )

            # ---------------- V^T = xn^T @ wV^T : [n, o] ----------------
            VT_bf = work.tile([128, 8, C], bf16, tag="VT")
            for g in range(4):  # two n-chunks per psum tile
                ps_v = psav.tile([128, 1024], fp32, tag="av")
                for half in range(2):
                    mc = 2 * g + half
                    for k in range(4):
                        nc.tensor.matmul(
                            ps_v[:, half * 512 : half * 512 + 512],
                            xn_r[:, k, mc * 128 : mc * 128 + 128],
                            w_r[:, k, 2 * C : 3 * C],
                            start=(k == 0),
                            stop=(k == 3),
                        )
                for half in range(2):
                    mc = 2 * g + half
                    nc.vector.tensor_tensor(
                        VT_bf[:, mc, :],
                        ps_v[:, half * 512 : half * 512 + 512],
                        vb_bc[:],
                        op=OP.add,
                    )

            # ---------------- Q, K : [o, n] ----------------
            QK_bf = work.tile([128, 8, N], bf16, tag="QK")
            for oc in (0, 4, 1, 5, 2, 6, 3, 7):
                ps_qk = pss.tile([128, 1024], fp32, tag="s")
                for nu in range(2):
                    for k in range(4):
                        nc.tensor.matmul(
                            ps_qk[:, nu * 512 : nu * 512 + 512],
                            w_r[:, k, oc * 128 : oc * 128 + 128],
                            xn_r[:, k, nu * 512 : nu * 512 + 512],
                            start=(k == 0),
                            stop=(k == 3),
                        )
                nc.vector.tensor_scalar(
                    QK_bf[:, oc, :],
                    ps_qk[:],
                    qb_sb[:, oc : oc + 1],
                    None,
                    op0=OP.add,
                )

            if debug:
                nc.sync.dma_start(dbg_xn.ap(), xn_r.bitcast(fp32)[:])
                nc.sync.dma_start(dbg_qk.ap(), QK_bf.bitcast(mybir.dt.uint16)[:])
                nc.sync.dma_start(dbg_vt.ap(), VT_bf.bitcast(mybir.dt.uint16)[:])

            # ---------------- attention, per head pair ----------------
            rr = work.tile([128, 4, 512], fp32, tag="rr")
            R_sb = work.tile([128, 4, N], fp32, tag="R")
            att = work.tile([128, 4, N], fp32, tag="att")

            for p in range(4):
                P_bf = ppool.tile([128, 8, 2048], bf16, tag="P")
                ps_av = psav.tile([128, 1024], fp32, tag="av")
                ps_r = psr.tile([128, 512], fp32, tag="r")
                nc.vector.memset(ps_r[:], 1.0)
                for s in range(8):
                    for nu in range(2):
                        ps_s = pss.tile([128, 1024], fp32, tag="s")
                        # S^T tiles: head A rows -> [:, 0:512], head B -> [:, 512:1024]
                        for e in range(2):  # head-in-pair, row-packed
                            nc.tensor.matmul(
                                ps_s[:, e * 512 : e * 512 + 512],
                                QK_bf[e * 64 : e * 64 + 64, 4 + p, s * 128 : s * 128 + 128],
                                QK_bf[e * 64 : e * 64 + 64, p, nu * 512 : nu * 512 + 512],
                                start=True,
                                stop=True,
                            )
                        # exp -> P slices (A at [0:1024], B at [1024:2048] within s row)
                        pview = P_bf[:, s, :].rearrange("p (h u n) -> p h u n", h=2, u=2)
                        nc.scalar.activation(
                            pview[:, :, nu, :],
                            ps_s[:].rearrange("p (h n) -> p h n", h=2),
                            AF.Exp,
                            scale=0.125,
                        )
                    # attnV: col-packed M=64 pair; accumulate over s
                    for e in range(2):
                        for nu in range(2):
                            nc.tensor.matmul(
                                ps_av[e * 64 : e * 64 + 64, nu * 512 : nu * 512 + 512],
                                VT_bf[:, s, p * 128 + e * 64 : p * 128 + e * 64 + 64],
                                P_bf[:, s, e * 1024 + nu * 512 : e * 1024 + nu * 512 + 512],
                                start=(s == 0),
                                stop=(s == 7),
                                skip_group_check=True,
                            )
                    # rowsums: M=1 ones matmuls at 4 col positions
                    for e in range(2):
                        for nu in range(2):
                            row = e * 64 + nu * 32
                            nc.tensor.matmul(
                                ps_r[row : row + 1, :],
                                ones_bf[:],
                                P_bf[:, s, e * 1024 + nu * 512 : e * 1024 + nu * 512 + 512],
                                start=(s == 0),
                                stop=(s == 7),
                                tile_position=(0, row),
                                skip_group_check=True,
                            )
                # reciprocal of rowsums (junk rows harmless)
                nc.vector.reciprocal(rr[:, p, :], ps_r[:])
                # broadcast to full partitions via DRAM roundtrip DMA
                for e in range(2):
                    for nu in range(2):
                        row = e * 64 + nu * 32
                        nc.sync.dma_start(
                            r8r_d.ap()[p, 2 * e + nu][None, :],
                            rr[row : row + 1, p, :],
                        )
                for e in range(2):
                    src_ap = r8r_d.ap()[p, 2 * e]
                    bcast = bass_mod.AP(
                        tensor=src_ap.tensor,
                        offset=src_ap.offset,
                        ap=[[0, 64], [512, 2], [1, 512]],
                    )
                    nc.sync.dma_start(
                        R_sb[e * 64 : e * 64 + 64, p, :].rearrange(
                            "q (u n) -> q u n", u=2
                        ),
                        bcast,
                    )
                if debug and p == 0:
                    nc.sync.dma_start(dbg_p0.ap(), P_bf.bitcast(mybir.dt.uint16)[:])
                # drain attnV accumulator (rounds to fp32r for the proj matmul)
                nc.vector.tensor_copy(att.bitcast(fp32r)[:, p, :], ps_av[:])
                # normalize in place -> fp32r
                nc.vector.tensor_tensor(
                    att.bitcast(fp32r)[:, p, :], att[:, p, :], R_sb[:, p, :], op=OP.mult
                )

            if debug:
                nc.sync.dma_start(dbg_rr.ap(), rr[:])
                nc.sync.dma_start(dbg_att.ap(), att[:])

            # ---------------- proj + residual ----------------
            att_r = att.bitcast(fp32r)
            for j in range(4):
                nc.vector.tensor_scalar(
                    x_sb[:, j, :], x_sb[:, j, :], pb_sb[:, j : j + 1], None, op0=OP.add
                )
            for oc in range(4):
                ps_o = pss.tile([128, 1024], fp32, tag="s")
                for nu in range(2):
                    for k in range(4):
                        nc.tensor.matmul(
                            ps_o[:, nu * 512 : nu * 512 + 512],
                            p_r[:, k, oc * 128 : oc * 128 + 128],
                            att_r[:, k, nu * 512 : nu * 512 + 512],
                            start=(k == 0),
                            stop=(k == 3),
                        )
                nc.vector.tensor_tensor(
                    x_sb[:, oc, :], ps_o[:], x_sb[:, oc, :], op=OP.add
                )
                nc.sync.dma_start(
                    y_d.ap().rearrange("(j p) n -> j p n", p=128)[oc], x_sb[:, oc, :]
                )

    nc.compile()
    return nc


def _get_nc(debug=False):
    key = "nc_dbg" if debug else "nc"
    if key not in _cache:
        _cache[key] = _build_bass(debug)
    return _cache[key]


def _host_inputs(x, norm_w, norm_b, qkv_w, qkv_b, proj_w, proj_b):
    x = np.asarray(x, dtype=np.float32).reshape(B, C, N)
    wqkvT = np.ascontiguousarray(np.asarray(qkv_w, dtype=np.float32).T)
    projT = np.ascontiguousarray(np.asarray(proj_w, dtype=np.float32).T)
    # G: [128, 4, 32] entries 1/16 where group(c) == g ; GT: [32, 4, 128] entries 1
    G = np.zeros((128, 4, 32), dtype=np.float32)
    GT = np.zeros((32, 4, 128), dtype=np.float32)
    for j in range(4):
        for p in range(128):
            g = 8 * j + p // 16
            G[p, j, g] = 1.0 / 16.0
            GT[g, j, p] = 1.0
    shared = {
        "wqkvT": wqkvT,
        "projT": projT,
        "qkv_b": np.asarray(qkv_b, dtype=np.float32),
        "proj_b": np.asarray(proj_b, dtype=np.float32),
        "norm_w": np.asarray(norm_w, dtype=np.float32),
        "norm_b": np.asarray(norm_b, dtype=np.float32),
        "Gmat": G,
        "GTmat": GT,
    }
    in_maps = [dict(shared, x=np.ascontiguousarray(x[i])) for i in range(B)]
    return in_maps


def kernel(x, norm_w, norm_b, qkv_w, qkv_b, proj_w, proj_b, _trace=False):
    from concourse import bass_utils

    nc = _get_nc()
    in_maps = _host_inputs(x, norm_w, norm_b, qkv_w, qkv_b, proj_w, proj_b)
    res = bass_utils.run_bass_kernel_spmd(
        nc, in_maps, core_ids=list(range(B)), trace=_trace
    )
    out = np.stack([res.results[i]["y"] for i in range(B)])
    _cache["last_result"] = res
    return out.reshape(B, C, 32, 32)


# revision 11
# speedup vs baseline: 1.1228x; 1.1228x over previous
"""Trainium2 Bass kernel for nn_AttentionBlock (GroupNorm + 8-head self-attention).

Data-parallel over batch: 8 batch elements -> 8 NeuronCores, one each.

Per-core layout ([c, n] with c on partitions, c = 4 chunks x 128, n = 1024):
  GroupNorm   : bn_stats per channel -> group-combine matmul (G) ->
                broadcast matmul (G^T) -> fused scale/shift -> xn (fp32r)
  QKV         : Q,K in [o, n] layout (lhsT = wqkvT chunks, fp32r),
                V in [n, o] layout (V^T, lhsT = xn chunks)
  Attention   : per head-pair p (heads 2p, 2p+1):
                  S^T[m,n] = K^T Q  (bf16, 2 heads row-packed via tile_position)
                  P = exp(S/8)      (ScalarE, PSUM->SBUF, bf16)
                  att_un = V^T.T @ P (bf16, 2 heads col-packed, M=64 each)
                  rowsums via ones-vector matmuls (M=1, 4 col positions)
                  normalize: att = att_un * (1/rowsum) broadcast via GpSimd
  Proj        : fp32r matmul + residual add, output fp32
"""

import numpy as np

NUM_GROUPS = 32
NUM_HEADS = 8
EPS = 1e-6
C = 512
N = 1024
B = 8

_cache = {}


def _build_bass(debug=False):
    import concourse.bacc as bacc
    import concourse.mybir as mybir
    import concourse.tile as tile

    fp32 = mybir.dt.float32
    fp32r = mybir.dt.float32r
    bf16 = mybir.dt.bfloat16
    AF = mybir.ActivationFunctionType
    OP = mybir.AluOpType
    AX = mybir.AxisListType

    nc = bacc.Bacc("TRN2", target_bir_lowering=False, debug=False)

    x_d = nc.dram_tensor("x", [C, N], fp32, kind="ExternalInput")
    wqkvT_d = nc.dram_tensor("wqkvT", [C, 3 * C], fp32, kind="ExternalInput")
    projT_d = nc.dram_tensor("projT", [C, C], fp32, kind="ExternalInput")
    qkvb_d = nc.dram_tensor("qkv_b", [3 * C], fp32, kind="ExternalInput")
    projb_d = nc.dram_tensor("proj_b", [C], fp32, kind="ExternalInput")
    nw_d = nc.dram_tensor("norm_w", [C], fp32, kind="ExternalInput")
    nb_d = nc.dram_tensor("norm_b", [C], fp32, kind="ExternalInput")
    G_d = nc.dram_tensor("Gmat", [128, 4, 32], fp32, kind="ExternalInput")
    GT_d = nc.dram_tensor("GTmat", [32, 4, 128], fp32, kind="ExternalInput")
    y_d = nc.dram_tensor("y", [C, N], fp32, kind="ExternalOutput")
    r8r_d = nc.dram_tensor("r8r_scratch", [4, 4, 512], fp32)
    if debug:
        dbg_xn = nc.dram_tensor("dbg_xn", [128, 4, N], fp32, kind="ExternalOutput")
        dbg_qk = nc.dram_tensor("dbg_qk", [128, 8, N], mybir.dt.uint16, kind="ExternalOutput")
        dbg_vt = nc.dram_tensor("dbg_vt", [128, 8, C], mybir.dt.uint16, kind="ExternalOutput")
        dbg_p0 = nc.dram_tensor("dbg_p0", [128, 8, 2048], mybir.dt.uint16, kind="ExternalOutput")
        dbg_rr = nc.dram_tensor("dbg_rr", [128, 4, 512], fp32, kind="ExternalOutput")
        dbg_att = nc.dram_tensor("dbg_att", [128, 4, N], fp32, kind="ExternalOutput")

    with tile.TileContext(nc) as tc:
        with (
            tc.tile_pool(name="const", bufs=1) as const,
            tc.tile_pool(name="work", bufs=1) as work,
            tc.tile_pool(name="ppool", bufs=1) as ppool,
            tc.tile_pool(name="rot", bufs=2) as rot,
            tc.tile_pool(name="pss", bufs=2, space="PSUM") as pss,     # 2x[128,1024]
            tc.tile_pool(name="psav", bufs=1, space="PSUM") as psav,   # [128,1024]
            tc.tile_pool(name="psr", bufs=1, space="PSUM") as psr,     # [128,512]
            tc.tile_pool(name="psg", bufs=1, space="PSUM") as psg,     # [128,512]
        ):
            # ---------------- load inputs ----------------
            x_sb = work.tile([128, 4, N], fp32, tag="x")
            nc.sync.dma_start(x_sb[:], x_d.ap().rearrange("(j p) n -> p j n", p=128))

            w_r = work.tile([128, 4, 3 * C], fp32r, tag="wr")
            p_r = work.tile([128, 4, C], fp32r, tag="pr")
            for j in range(4):
                stg = rot.tile([128, 3 * C], fp32, tag="stage")
                nc.sync.dma_start(
                    stg[:], wqkvT_d.ap().rearrange("(j p) o -> j p o", p=128)[j]
                )
                nc.scalar.copy(w_r[:, j, :], stg[:])
            for j in range(4):
                stg = rot.tile([128, C], fp32, tag="stage2")
                nc.sync.dma_start(
                    stg[:], projT_d.ap().rearrange("(j p) o -> j p o", p=128)[j]
                )
                nc.scalar.copy(p_r[:, j, :], stg[:])

            G_sb = const.tile([128, 4, 32], fp32, tag="G")
            GT_sb = const.tile([32, 4, 128], fp32, tag="GT")
            nc.sync.dma_start(G_sb[:], G_d.ap())
            nc.sync.dma_start(GT_sb[:], GT_d.ap())
            nw_sb = const.tile([128, 4], fp32, tag="nw")
            nb_sb = const.tile([128, 4], fp32, tag="nb")
            nc.sync.dma_start(nw_sb[:], nw_d.ap().rearrange("(j p) -> p j", p=128))
            nc.sync.dma_start(nb_sb[:], nb_d.ap().rearrange("(j p) -> p j", p=128))
            qb_sb = const.tile([128, 8], fp32, tag="qb")
            nc.sync.dma_start(
                qb_sb[:], qkvb_d.ap()[0 : 2 * C].rearrange("(o p) -> p o", p=128)
            )
            pb_sb = const.tile([128, 4], fp32, tag="pb")
            nc.sync.dma_start(pb_sb[:], projb_d.ap().rearrange("(j p) -> p j", p=128))
            import concourse.bass as bass_mod
            vb_src = qkvb_d.ap()[2 * C : 3 * C]
            vb_bcast_ap = bass_mod.AP(
                tensor=vb_src.tensor,
                offset=vb_src.offset,
                ap=[[0, 128], [1, C]],
            )
            vb_bc = const.tile([128, C], fp32, tag="vbbc")
            nc.sync.dma_start(vb_bc[:], vb_bcast_ap)
            ones_bf = const.tile([128, 1], bf16, tag="ones")
            nc.vector.memset(ones_bf[:], 1.0)

            # ---------------- groupnorm ----------------
            stats = work.tile([128, 4, 2, 6], fp32, tag="stats")
            for j in range(4):
                for u in range(2):
                    nc.vector.bn_stats(
                        stats[:, j, u, :], x_sb[:, j, u * 512 : u * 512 + 512]
                    )
            mv = work.tile([128, 4, 2], fp32, tag="mv")
            for j in range(4):
                nc.vector.bn_aggr(mv[:, j, :], stats[:, j, :, :])
            # ssq = (mean, E[x^2]) per channel
            ssq = work.tile([128, 4, 2], fp32, tag="ssq")
            nc.vector.tensor_copy(ssq[:, :, 0], mv[:, :, 0])
            nc.vector.tensor_tensor(
                ssq[:, :, 1], mv[:, :, 0], mv[:, :, 0], op=OP.mult
            )
            nc.vector.tensor_tensor(
                ssq[:, :, 1], ssq[:, :, 1], mv[:, :, 1], op=OP.add
            )
            # group stats [32, 2] = (mu_g, E[x^2]_g); G has 1/16 entries
            ps_g = psg.tile([32, 2], fp32, tag="g")
            for j in range(4):
                nc.tensor.matmul(
                    ps_g[:], G_sb[:, j, :], ssq[:, j, :], start=(j == 0), stop=(j == 3)
                )
            st2 = work.tile([32, 2], fp32, tag="st2")
            # var = E[x^2] - mu^2 -> rstd = 1/sqrt(var+eps); st2 = (mu, rstd)
            nc.vector.tensor_copy(st2[:, 0:1], ps_g[:, 0:1])
            var = work.tile([32, 1], fp32, tag="var")
            nc.vector.tensor_tensor(var[:], st2[:, 0:1], st2[:, 0:1], op=OP.mult)
            nc.vector.tensor_tensor(var[:], ps_g[:, 1:2], var[:], op=OP.subtract)
            eps_sb = const.tile([32, 1], fp32, tag="eps")
            nc.vector.memset(eps_sb[:], float(EPS))
            nc.scalar.activation(var[:], var[:], AF.Sqrt, bias=eps_sb[:], scale=1.0)
            nc.vector.reciprocal(st2[:, 1:2], var[:])
            # broadcast back to channels: [128, 2] per chunk
            ps_bc = psg.tile([128, 4, 2], fp32, tag="g")
            for j in range(4):
                nc.tensor.matmul(
                    ps_bc[:, j, :], GT_sb[:, j, :], st2[:], start=True, stop=True
                )
            # alpha = rstd*w ; beta = b - mu*alpha
            ab = work.tile([128, 4, 2], fp32, tag="ab")
            nc.vector.tensor_tensor(ab[:, :, 0], ps_bc[:, :, 1], nw_sb[:], op=OP.mult)
            nc.vector.tensor_tensor(ab[:, :, 1], ps_bc[:, :, 0], ab[:, :, 0], op=OP.mult)
            nc.vector.tensor_tensor(ab[:, :, 1], nb_sb[:], ab[:, :, 1], op=OP.subtract)

            xn_r = work.tile([128, 4, N], fp32r, tag="xn")
            for j in range(4):
                nc.vector.tensor_scalar(
                    xn_r[:, j, :],
                    x_sb[:, j, :],
                    ab[:, j, 0:1],
                    ab[:, j, 1:2],
                    op0=OP.mult,
                    op1=OP.add,
                )

            # ---------------- V^T = xn^T @ wV^T : [n, o] ----------------
            VT_bf = work.tile([128, 8, C], bf16, tag="VT")
            for g in range(4):  # two n-chunks per psum tile
                ps_v = psav.tile([128, 1024], fp32, tag="av")
                for half in range(2):
                    mc = 2 * g + half
                    for k in range(4):
                        nc.tensor.matmul(
                            ps_v[:, half * 512 : half * 512 + 512],
                            xn_r[:, k, mc * 128 : mc * 128 + 128],
                            w_r[:, k, 2 * C : 3 * C],
                            start=(k == 0),
                            stop=(k == 3),
                        )
                for half in range(2):
                    mc = 2 * g + half
                    nc.vector.tensor_tensor(
                        VT_bf[:, mc, :],
                        ps_v[:, half * 512 : half * 512 + 512],
                        vb_bc[:],
                        op=OP.add,
                    )

            # ---------------- Q, K : [o, n] ----------------
            QK_bf = work.tile([128, 8, N], bf16, tag="QK")
            for oc in (0, 4, 1, 5, 2, 6, 3, 7):
                ps_qk = pss.tile([128, 1024], fp32, tag="s")
                for nu in range(2):
                    for k in range(4):
                        nc.tensor.matmul(
                            ps_qk[:, nu * 512 : nu * 512 + 512],
                            w_r[:, k, oc * 128 : oc * 128 + 128],
                            xn_r[:, k, nu * 512 : nu * 512 + 512],
                            start=(k == 0),
                            stop=(k == 3),
                        )
                nc.scalar.add(QK_bf[:, oc, :], ps_qk[:], qb_sb[:, oc : oc + 1])

            if debug:
                nc.sync.dma_start(dbg_xn.ap(), xn_r.bitcast(fp32)[:])
                nc.sync.dma_start(dbg_qk.ap(), QK_bf.bitcast(mybir.dt.uint16)[:])
                nc.sync.dma_start(dbg_vt.ap(), VT_bf.bitcast(mybir.dt.uint16)[:])

            # ---------------- attention, per head pair ----------------
            rr = work.tile([128, 4, 512], fp32, tag="rr")
            R_sb = work.tile([128, 4, N], fp32, tag="R")
            att = work.tile([128, 4, N], fp32, tag="att")

            for p in range(4):
                P_bf = ppool.tile([128, 8, 2048], bf16, tag="P")
                ps_av = psav.tile([128, 1024], fp32, tag="av")
                ps_r = psr.tile([128, 512], fp32, tag="r")
                nc.vector.memset(ps_r[:], 1.0)
                for s in range(8):
                    for nu in range(2):
                        ps_s = pss.tile([128, 1024], fp32, tag="s")
                        # S^T tiles: head A rows -> [:, 0:512], head B -> [:, 512:1024]
                        for e in range(2):  # head-in-pair, row-packed
                            nc.tensor.matmul(
                                ps_s[:, e * 512 : e * 512 + 512],
                                QK_bf[e * 64 : e * 64 + 64, 4 + p, s * 128 : s * 128 + 128],
                                QK_bf[e * 64 : e * 64 + 64, p, nu * 512 : nu * 512 + 512],
                                start=True,
                                stop=True,
                            )
                        # exp -> P slices (A at [0:1024], B at [1024:2048] within s row)
                        pview = P_bf[:, s, :].rearrange("p (h u n) -> p h u n", h=2, u=2)
                        nc.scalar.activation(
                            pview[:, :, nu, :],
                            ps_s[:].rearrange("p (h n) -> p h n", h=2),
                            AF.Exp,
                            scale=0.125,
                        )
                    # rowsums: M=1 ones matmuls, 4 col positions packed
                    for e in range(2):
                        for nu in range(2):
                            row = e * 64 + nu * 32
                            nc.tensor.matmul(
                                ps_r[row : row + 1, :],
                                ones_bf[:],
                                P_bf[:, s, e * 1024 + nu * 512 : e * 1024 + nu * 512 + 512],
                                start=(s == 0),
                                stop=(s == 7),
                                tile_position=(0, row),
                                skip_group_check=True,
                            )
                    # attnV: col-packed M=64 pairs (nu-outer so A/B adjacent)
                    for nu in range(2):
                        for e in range(2):
                            nc.tensor.matmul(
                                ps_av[e * 64 : e * 64 + 64, nu * 512 : nu * 512 + 512],
                                VT_bf[:, s, p * 128 + e * 64 : p * 128 + e * 64 + 64],
                                P_bf[:, s, e * 1024 + nu * 512 : e * 1024 + nu * 512 + 512],
                                start=(s == 0),
                                stop=(s == 7),
                                skip_group_check=True,
                            )
                # reciprocal of rowsums (junk rows harmless; approx ~4e-6 rel)
                nc.vector.reciprocal_approx_fast(rr[:, p, :], ps_r[:])
                # broadcast to full partitions via DRAM roundtrip DMA
                for e in range(2):
                    for nu in range(2):
                        row = e * 64 + nu * 32
                        nc.sync.dma_start(
                            r8r_d.ap()[p, 2 * e + nu][None, :],
                            rr[row : row + 1, p, :],
                        )
                for e in range(2):
                    src_ap = r8r_d.ap()[p, 2 * e]
                    bcast = bass_mod.AP(
                        tensor=src_ap.tensor,
                        offset=src_ap.offset,
                        ap=[[0, 64], [512, 2], [1, 512]],
                    )
                    nc.sync.dma_start(
                        R_sb[e * 64 : e * 64 + 64, p, :].rearrange(
                            "q (u n) -> q u n", u=2
                        ),
                        bcast,
                    )
                if debug and p == 0:
                    nc.sync.dma_start(dbg_p0.ap(), P_bf.bitcast(mybir.dt.uint16)[:])
                # drain attnV accumulator (rounds to fp32r for the proj matmul)
                nc.vector.tensor_copy(att.bitcast(fp32r)[:, p, :], ps_av[:])
                # normalize in place -> fp32r
                nc.vector.tensor_tensor(
                    att.bitcast(fp32r)[:, p, :], att[:, p, :], R_sb[:, p, :], op=OP.mult
                )

            if debug:
                nc.sync.dma_start(dbg_rr.ap(), rr[:])
                nc.sync.dma_start(dbg_att.ap(), att[:])

            # ---------------- proj + residual ----------------
            att_r = att.bitcast(fp32r)
            for j in range(4):
                nc.vector.tensor_scalar(
                    x_sb[:, j, :], x_sb[:, j, :], pb_sb[:, j : j + 1], None, op0=OP.add
                )
            for oc in range(4):
                ps_o = pss.tile([128, 1024], fp32, tag="s")
                for nu in range(2):
                    for k in range(4):
                        nc.tensor.matmul(
                            ps_o[:, nu * 512 : nu * 512 + 512],
                            p_r[:, k, oc * 128 : oc * 128 + 128],
                            att_r[:, k, nu * 512 : nu * 512 + 512],
                            start=(k == 0),
                            stop=(k == 3),
                        )
                nc.vector.tensor_tensor(
                    x_sb[:, oc, :], ps_o[:], x_sb[:, oc, :], op=OP.add
                )
                nc.sync.dma_start(
                    y_d.ap().rearrange("(j p) n -> j p n", p=128)[oc], x_sb[:, oc, :]
                )

    nc.compile()
    return nc


def _get_nc(debug=False):
    key = "nc_dbg" if debug else "nc"
    if key not in _cache:
        _cache[key] = _build_bass(debug)
    return _cache[key]


def _host_inputs(x, norm_w, norm_b, qkv_w, qkv_b, proj_w, proj_b):
    x = np.asarray(x, dtype=np.float32).reshape(B, C, N)
    wqkvT = np.ascontiguousarray(np.asarray(qkv_w, dtype=np.float32).T)
    projT = np.ascontiguousarray(np.asarray(proj_w, dtype=np.float32).T)
    # G: [128, 4, 32] entries 1/16 where group(c) == g ; GT: [32, 4, 128] entries 1
    G = np.zeros((128, 4, 32), dtype=np.float32)
    GT = np.zeros((32, 4, 128), dtype=np.float32)
    for j in range(4):
        for p in range(128):
            g = 8 * j + p // 16
            G[p, j, g] = 1.0 / 16.0
            GT[g, j, p] = 1.0
    shared = {
        "wqkvT": wqkvT,
        "projT": projT,
        "qkv_b": np.asarray(qkv_b, dtype=np.float32),
        "proj_b": np.asarray(proj_b, dtype=np.float32),
        "norm_w": np.asarray(norm_w, dtype=np.float32),
        "norm_b": np.asarray(norm_b, dtype=np.float32),
        "Gmat": G,
        "GTmat": GT,
    }
    in_maps = [dict(shared, x=np.ascontiguousarray(x[i])) for i in range(B)]
    return in_maps


def kernel(x, norm_w, norm_b, qkv_w, qkv_b, proj_w, proj_b, _trace=False):
    from concourse import bass_utils

    nc = _get_nc()
    in_maps = _host_inputs(x, norm_w, norm_b, qkv_w, qkv_b, proj_w, proj_b)
    res = bass_utils.run_bass_kernel_spmd(
        nc, in_maps, core_ids=list(range(B)), trace=_trace
    )
    out = np.stack([res.results[i]["y"] for i in range(B)])
    _cache["last_result"] = res
    return out.reshape(B, C, 32, 32)
